# revision 23
# baseline (speedup 1.0000x reference)
"""GCN 2-layer message-passing kernel for Trainium2 (8 NeuronCores, Bass/Tile).

Strategy (graph/data parallel per the sharding hint):
  - Nodes partitioned into 8 contiguous ranges (6250 per core).
  - Host does INTEGER/index prep only: add self-loops, bucket edges by
    destination core/block-of-128, sort by source, build gather-index +
    dst-slot metadata, integer in-degree counts. All floating-point math
    runs on device.
  - Per layer: each core computes g = dinv * (x @ W) rows for its own
    nodes (PE matmul + ACT per-partition scale, cast to bf16), AllGather
    of the bf16 g-table across the 8 cores (halo exchange). Then, per
    chunk of destination blocks, one dma_gather instruction fetches all
    message rows (g[src]) for the chunk's edge tiles; per 128-edge tile a
    0/1 selection matrix (DVE is_equal against an iota matrix) scatter-
    reduces the messages into the block's PSUM accumulator via one PE
    matmul; the block output is
      relu(dinv * segsum + x @ res_w + conv_b + res_b)
    with bias via a K=1 outer-product matmul and residual accumulated in
    a second PSUM bank; dst-degree scaling via ACT per-partition scale.
  - dma_gather uses int16 indices, so the gather table is addressed in a
    low half (rows < 32768) and a high half; each block's edges are
    split into low/high tile groups (host-side integer split).

kernel(**inputs) takes FULL inputs and returns the FULL [50000, 128]
float32 output.
"""
import sys
from contextlib import ExitStack

import numpy as np

if '/opt/trn_rl_repo' not in sys.path:
    sys.path.insert(0, '/opt/trn_rl_repo')

import ml_dtypes

from concourse import bacc, mybir, tile
from concourse.bass_utils import run_bass_kernel_spmd
from concourse.vector_clock import ScopedClock


def _patched_drain_and_barrier(self, tick_clock, wait_clock):
    """Split the kernel-tail drain's sem waits across single-wait drains:
    walrus's NO_STRUCT codegen rejects >1 sync wait on InstDrain."""
    drain_inst = self.nc.sync.drain()
    wait_clock.add_sem_waits(drain_inst.ins,
                             ScopedClock({None: tick_clock.global_clock}))
    si = drain_inst.ins.sync_info
    if si is not None and si.on_wait is not None and len(si.on_wait) > 1:
        waits = list(si.on_wait)
        del si.on_wait[1:]
        for w in waits[1:]:
            d2 = self.nc.sync.drain()
            si2 = d2.ins.sync_info
            if si2 is None:
                d2.ins.sync_info = mybir.SyncInfo(on_wait=[w], on_update=[])
            else:
                si2.on_wait.append(w)
    self.nc.all_engine_barrier()
    assert self.sems is not None
    popped = self.nc._tile_sem_poison_stack.pop()
    assert popped is self._sem_poison
    self.nc.clear_and_free_semaphores(list(self.sems.allocated().values()))
    self.nc.all_engine_barrier()


tile.TileContext._drain_and_barrier = _patched_drain_and_barrier


def split_sync_waits(nc, max_waits=1):
    """Walrus codegen rejects >1 sync wait on several instruction encodings.
    Hoist excess waits onto same-engine no-ops placed just before."""
    import bass_rust
    try:
        funcs = list(nc.m.functions)
    except Exception:
        funcs = [nc.main_func]
    seen = 0
    for fn in funcs:
        for bb in fn.blocks:
            insts = bb.instructions
            new = []
            for ins in insts:
                si = ins.sync_info
                if si is not None and si.on_wait and len(si.on_wait) > max_waits:
                    waits = list(si.on_wait)
                    extra, keep = waits[:-max_waits], waits[-max_waits:]
                    for w in extra:
                        nop = bass_rust.InstNoOp(
                            name=f"I-waitsplit-{seen}", ins=[], outs=[])
                        seen += 1
                        nop.engine = ins.engine
                        nop.sync_info = mybir.SyncInfo(on_wait=[w], on_update=[])
                        new.append(nop)
                    del si.on_wait[:]
                    si.on_wait.extend(keep)
                new.append(ins)
            insts[:] = new
    return seen


bf16 = ml_dtypes.bfloat16
P = 128          # partitions / tile edge
C = 8            # cores
D = 128          # hidden dim
HI = 32768       # int16 index reach of dma_gather
CB = 4           # dst blocks per gather chunk


# ---------------------------------------------------------------------------
# Host-side integer/index prep (sharding + metadata; no FP math on values)
# ---------------------------------------------------------------------------

def prep(edge_index, n_nodes):
    N = n_nodes
    npc = N // C
    assert npc * C == N
    B = (npc + P - 1) // P
    npad = B * P

    ei = np.asarray(edge_index)
    # self-loops are handled on-device via an identity matmul per block;
    # they still count toward the degree.
    src_all = ei[0].astype(np.int64)
    dst_all = ei[1].astype(np.int64)
    deg_all = np.bincount(dst_all, minlength=N) + 1

    own_s = src_all // npc
    local = src_all - own_s * npc
    # piece-major table layout: piece p = blocks [PBH[p], PBH[p+1]) of every
    # core, so each piece's AllGather lands contiguously in the table.
    PBH = [0, 13, 25, 37, B]
    pb_rows = [(PBH[p + 1] - PBH[p]) * P for p in range(4)]
    base = np.concatenate([[0], np.cumsum([C * r for r in pb_rows])])
    blk_of = local >> 7
    piece = np.searchsorted(np.array(PBH[1:]), blk_of, side='right')
    row_all = (base[piece] + own_s * np.array(pb_rows)[piece]
               + (local - np.array(PBH)[piece] * P))

    owner_all = dst_all // npc
    per_core = []
    nlo = np.zeros((C, B), dtype=np.int64)
    nhi = np.zeros((C, B), dtype=np.int64)
    for c in range(C):
        m = owner_all == c
        r = row_all[m]
        dloc = dst_all[m] - c * npc
        blk = dloc >> 7
        slot = dloc & 127
        hi = (r >= HI).astype(np.int64)
        # sort by (block, hi, row) so each block = [lo edges..., hi edges...]
        order = np.lexsort((r, hi, blk))
        r, blk, slot, hi = r[order], blk[order], slot[order], hi[order]
        per_core.append((r, blk, slot))
        for b in range(B):
            mb = blk == b
            nhi[c, b] = (hi[mb]).sum()
            nlo[c, b] = mb.sum() - nhi[c, b]

    T_lo = np.maximum((nlo.max(axis=0) + P - 1) // P, 1)
    T_hi = (nhi.max(axis=0) + P - 1) // P            # may be 0 for a block
    T_b = (T_lo + T_hi).astype(np.int64)
    T_total = int(T_b.sum())

    # per-block tile layout: T_lo[b] low tiles then T_hi[b] high tiles
    tile_base = np.concatenate([[0], np.cumsum(T_b)])
    # low/high gather index sequences (tile-major, per core)
    n_lo_total = int(T_lo.sum()) * P
    n_hi_total = int(T_hi.sum()) * P
    lo_base = np.concatenate([[0], np.cumsum(T_lo)])   # in tiles
    hi_base = np.concatenate([[0], np.cumsum(T_hi)])

    slots = np.full((C, T_total * P), -1.0, dtype=np.float32)
    idx_lo = np.zeros((C, n_lo_total), dtype=np.int64)
    idx_hi = np.zeros((C, max(n_hi_total, 16)), dtype=np.int64)
    for c in range(C):
        r, blk, slot = per_core[c]
        bstart = np.concatenate([[0], np.cumsum(nlo[c] + nhi[c])])
        for b in range(B):
            e0, e1 = bstart[b], bstart[b + 1]
            k_lo = int(nlo[c, b])
            rl, sl = r[e0:e0 + k_lo], slot[e0:e0 + k_lo]
            rh, sh = r[e0 + k_lo:e1], slot[e0 + k_lo:e1]
            o = lo_base[b] * P
            idx_lo[c, o:o + k_lo] = rl
            o = hi_base[b] * P
            idx_hi[c, o:o + len(rh)] = rh - HI
            o = tile_base[b] * P
            slots[c, o:o + k_lo] = sl
            o2 = (tile_base[b] + T_lo[b]) * P
            slots[c, o2:o2 + len(sh)] = sh

    deg = np.ones((C, P, B), dtype=np.float32)
    sdeg_row = np.ones((C, 1, npad), dtype=np.float32)
    for c in range(C):
        dpad = np.ones(npad, dtype=np.float32)
        dpad[:npc] = deg_all[c * npc:(c + 1) * npc].astype(np.float32)
        deg[c] = dpad.reshape(B, P).T
        sdeg_row[c, 0] = np.sqrt(dpad)

    def pack16(a):
        # wrapped layout: element j -> [j % 16, j // 16], replicated to the
        # 8 Q7 cores' partition groups (128 partitions total)
        n = a.shape[1]
        w = a.reshape(a.shape[0], n // 16, 16).transpose(0, 2, 1).astype(np.int16)
        return np.tile(w, (1, 8, 1)).copy()

    # chunking of blocks for gather calls
    chunks = []
    for b0 in range(0, B, CB):
        b1 = min(b0 + CB, B)
        chunks.append(dict(
            b0=b0, b1=b1,
            lo_t0=int(lo_base[b0]), lo_t1=int(lo_base[b1]),
            hi_t0=int(hi_base[b0]), hi_t1=int(hi_base[b1]),
        ))

    return dict(
        npc=npc, npad=npad, B=B,
        T_lo=T_lo.tolist(), T_hi=T_hi.tolist(), T_b=T_b.tolist(),
        tile_base=tile_base.tolist(), T_total=T_total,
        n_lo16=n_lo_total // 16, n_hi16=max(n_hi_total, 16) // 16,
        chunks=chunks,
        idx_lo=pack16(idx_lo), idx_hi=pack16(idx_hi),
        slots=slots.reshape(C, T_total, P).transpose(0, 2, 1).copy(),
        deg=deg, sdeg_row=sdeg_row,
    )


# ---------------------------------------------------------------------------
# Device program (uniform across the 8 cores)
# ---------------------------------------------------------------------------

def build_program(meta):
    npad, B, T_total = meta['npad'], meta['B'], meta['T_total']
    T_lo, T_hi, tile_base = meta['T_lo'], meta['T_hi'], meta['tile_base']
    chunks = meta['chunks']
    TBL = C * npad
    f32 = mybir.dt.float32
    bf = mybir.dt.bfloat16
    max_lo_tiles = max(ch['lo_t1'] - ch['lo_t0'] for ch in chunks)
    max_hi_tiles = max(ch['hi_t1'] - ch['hi_t0'] for ch in chunks)

    nc = bacc.Bacc(None, target_bir_lowering=False, num_swdge_queues=4)
    xT_p = nc.declare_dram_parameter("xT", [P, npad], f32, isOutput=False)
    w1_p = nc.declare_dram_parameter("w1", [P, D], f32, isOutput=False)
    w2_p = nc.declare_dram_parameter("w2", [P, D], f32, isOutput=False)
    rw_p = nc.declare_dram_parameter("resw", [P, D], f32, isOutput=False)
    cb_p = nc.declare_dram_parameter("convb", [2, D], f32, isOutput=False)
    rb_p = nc.declare_dram_parameter("resb", [1, D], f32, isOutput=False)
    deg_p = nc.declare_dram_parameter("deg", [P, B], f32, isOutput=False)
    ilo_p = nc.declare_dram_parameter("idx_lo", [128, meta['n_lo16']], mybir.dt.int16, isOutput=False)
    ihi_p = nc.declare_dram_parameter("idx_hi", [128, meta['n_hi16']], mybir.dt.int16, isOutput=False)
    slot_p = nc.declare_dram_parameter("slot", [P, T_total], bf, isOutput=False)
    iota_p = nc.declare_dram_parameter("iota", [P, P], bf, isOutput=False)
    ident_p = nc.declare_dram_parameter("ident", [P, P], f32, isOutput=False)
    ones_p = nc.declare_dram_parameter("ones", [1, D], bf, isOutput=False)
    out_p = nc.declare_dram_parameter("out", [npad, D], f32, isOutput=True)

    # AllGather in 4 pieces so transfers overlap compute; piece tensors give
    # tile exact deps, outs land strided in the single gather table.
    PB = [0, 13, 25, 37, B]
    NPIECE = 4
    g1p = [nc.dram_tensor(f"g1o{p}", [(PB[p + 1] - PB[p]) * P, D], bf)
           for p in range(NPIECE)]
    g2p = [nc.dram_tensor(f"g2o{p}", [(PB[p + 1] - PB[p]) * P, D], bf)
           for p in range(NPIECE)]
    g1_full = nc.dram_tensor("g1_full", [TBL, D], bf, addr_space="Shared")
    g2_full = nc.dram_tensor("g2_full", [TBL, D], bf, addr_space="Shared")

    def piece_of(b):
        for p in range(NPIECE):
            if PB[p] <= b < PB[p + 1]:
                return p, b - PB[p]
        raise AssertionError

    pb_rows = [(PB[p + 1] - PB[p]) * P for p in range(NPIECE)]
    pbase = [0]
    for p in range(NPIECE):
        pbase.append(pbase[-1] + C * pb_rows[p])

    def ag_piece(tag, gp_list, g_full, p):
        with nc.named_scope(f"{tag}_{p}"):
            nc.gpsimd.collective_compute(
                "AllGather", mybir.AluOpType.bypass,
                replica_groups=[list(range(C))],
                ins=[gp_list[p][:, :]],
                outs=[g_full[pbase[p]:pbase[p + 1], :]])

    with tile.TileContext(nc) as tc, ExitStack() as ctx:
        const = ctx.enter_context(tc.tile_pool(name="const", bufs=1))
        gbufL = ctx.enter_context(tc.tile_pool(name="gbufL", bufs=6))
        gbufH = ctx.enter_context(tc.tile_pool(name="gbufH", bufs=3))
        work = ctx.enter_context(tc.tile_pool(name="work", bufs=6))
        outp = ctx.enter_context(tc.tile_pool(name="outp", bufs=3))
        psum = ctx.enter_context(tc.tile_pool(name="psum", bufs=2, space="PSUM"))

        # ---- constants / persistent state ----
        xT = const.tile([P, npad], bf)
        for k0 in range(0, npad, 784):
            xstg = outp.tile([P, 784], f32, tag="xstg")
            nc.sync.dma_start(out=xstg[:], in_=xT_p[:, k0:k0 + 784])
            nc.vector.tensor_copy(out=xT[:, k0:k0 + 784], in_=xstg[:])
        x1T = const.tile([P, npad], bf)          # layer-1 output, transposed
        ones1 = const.tile([1, D], bf)
        nc.sync.dma_start(out=ones1[:], in_=ones_p[:, :])
        w1f = const.tile([P, D], f32)
        nc.sync.dma_start(out=w1f[:], in_=w1_p[:, :])
        w1 = const.tile([P, D], bf)
        nc.vector.tensor_copy(out=w1[:], in_=w1f[:])
        w2f = const.tile([P, D], f32)
        nc.sync.dma_start(out=w2f[:], in_=w2_p[:, :])
        rwf = const.tile([P, D], f32)
        nc.sync.dma_start(out=rwf[:], in_=rw_p[:, :])
        w2b = const.tile([P, D], bf)
        nc.vector.tensor_copy(out=w2b[:], in_=w2f[:])
        rwb = const.tile([P, D], bf)
        nc.vector.tensor_copy(out=rwb[:], in_=rwf[:])
        # SBUF-resident scaled g tables for the self-loop contribution
        g1sb = const.tile([P, B * D], bf)
        g2sb = const.tile([P, B * D], bf)

        rb = const.tile([1, D], f32)
        nc.sync.dma_start(out=rb[:], in_=rb_p[:, :])
        bcomb = []
        for l in range(2):
            cbl = const.tile([1, D], f32, tag=f"cb{l}")
            nc.sync.dma_start(out=cbl[:], in_=cb_p[l:l + 1, :])
            bc = const.tile([1, D], bf, tag=f"bcomb{l}")
            nc.vector.tensor_tensor(out=bc[:], in0=cbl[:], in1=rb[:],
                                    op=mybir.AluOpType.add)
            bcomb.append(bc)

        iota = const.tile([P, P], bf)
        nc.sync.dma_start(out=iota[:], in_=iota_p[:, :])
        ident = const.tile([P, P], f32)
        nc.sync.dma_start(out=ident[:], in_=ident_p[:, :])
        ident_bf = const.tile([P, P], bf)
        nc.vector.tensor_copy(out=ident_bf[:], in_=ident[:])

        ilo = const.tile([128, meta['n_lo16']], mybir.dt.int16)
        nc.sync.dma_start(out=ilo[:], in_=ilo_p[:, :])
        ihi = const.tile([128, meta['n_hi16']], mybir.dt.int16)
        nc.sync.dma_start(out=ihi[:], in_=ihi_p[:, :])
        slots = const.tile([P, T_total], bf)
        nc.sync.dma_start(out=slots[:], in_=slot_p[:, :])

        degt = const.tile([P, B], f32)
        nc.sync.dma_start(out=degt[:], in_=deg_p[:, :])
        sdeg = const.tile([P, B], f32)
        nc.scalar.activation(out=sdeg[:], in_=degt[:],
                             func=mybir.ActivationFunctionType.Sqrt)
        dinv = const.tile([P, B], f32)
        nc.vector.reciprocal(out=dinv[:], in_=sdeg[:])


        # ---- phase 1: g1 = dinv * (x @ W1) for own rows, then AllGather ----
        with nc.named_scope("phase1"):
            for b in range(B):
                cs = slice(b * P, (b + 1) * P)
                ds = slice(b * D, (b + 1) * D)
                ph = psum.tile([P, D], f32, tag="ph")
                nc.tensor.matmul(out=ph[:], lhsT=xT[:, cs], rhs=w1[:],
                                 start=True, stop=True)
                nc.scalar.activation(out=g1sb[:, ds], in_=ph[:],
                                     func=mybir.ActivationFunctionType.Copy,
                                     scale=dinv[:, b:b + 1])
                p, lb = piece_of(b)
                nc.sync.dma_start(out=g1p[p][lb * P:(lb + 1) * P, :],
                                  in_=g1sb[:, ds])
                if b == PB[p + 1] - 1:
                    ag_piece("ag1", g1p, g1_full, p)

        # greedy queue balancing: 4 SWDGE queues = 4 independent Q7 core
        # pairs generating gather descriptors in parallel
        qload = [0, 0, 0, 0]

        def pick_queue(nidx):
            q = qload.index(min(qload))
            qload[q] += nidx
            return q

        def layer(l, g_full, g_own_l):
            selmax = max(T_lo[b] + T_hi[b] for b in range(B))
            half_lo = (max_lo_tiles + 1) // 2
            # queue position of each ag2 piece: ~3 chunks after its last block
            # is produced, so the Pool queue never stalls waiting for it
            ag2_emitted = set()
            ag2_at = {}
            if l == 0:
                for p in range(NPIECE):
                    ag2_at.setdefault((PB[p + 1] - 1) // CB + 2, []).append(p)
            for ci, ch in enumerate(chunks):
                for p in ag2_at.get(ci, []):
                    ag_piece("ag2", g2p, g2_full, p)
                    ag2_emitted.add(p)
                nlo_t = ch['lo_t1'] - ch['lo_t0']
                nhi_t = ch['hi_t1'] - ch['hi_t0']
                # split the lo gather across two queues for deeper overlap
                ta = (nlo_t + 1) // 2
                tb = nlo_t - ta
                gloa = gbufL.tile([P, half_lo, D], bf, tag="gloa")
                nc.gpsimd.dma_gather(
                    out_ap=gloa[:, :ta, :], in_ap=g_full[:, :],
                    idxs_ap=ilo[:, ch['lo_t0'] * 8:(ch['lo_t0'] + ta) * 8],
                    num_idxs=ta * P, num_idxs_reg=ta * P, elem_size=D,
                    single_packet=False, queue_num=pick_queue(ta * P))
                glob = gbufL.tile([P, half_lo, D], bf, tag="glob")
                if tb > 0:
                    nc.gpsimd.dma_gather(
                        out_ap=glob[:, :tb, :], in_ap=g_full[:, :],
                        idxs_ap=ilo[:, (ch['lo_t0'] + ta) * 8:ch['lo_t1'] * 8],
                        num_idxs=tb * P, num_idxs_reg=tb * P, elem_size=D,
                        single_packet=False, queue_num=pick_queue(tb * P))
                if nhi_t > 0:
                    ghi = gbufH.tile([P, max(max_hi_tiles, 1), D], bf, tag="ghi")
                    nc.gpsimd.dma_gather(
                        out_ap=ghi[:, :nhi_t, :], in_ap=g_full[HI:, :],
                        idxs_ap=ihi[:, ch['hi_t0'] * 8:ch['hi_t1'] * 8],
                        num_idxs=nhi_t * P, num_idxs_reg=nhi_t * P, elem_size=D,
                        single_packet=False, queue_num=pick_queue(nhi_t * P))
                for b in range(ch['b0'], ch['b1']):
                    cs = slice(b * P, (b + 1) * P)
                    # sqrt(deg)-scaled bias + residual in their own bank so
                    # the PE can run them independently of the scatter chain
                    pB = psum.tile([P, D], f32, tag="pB")
                    nc.tensor.matmul(out=pB[:], lhsT=ones1[:],
                                     rhs=bcomb[l][:], start=True, stop=False)
                    nc.tensor.matmul(out=pB[:],
                                     lhsT=(xT if l == 0 else x1T)[:, cs],
                                     rhs=rwb[:], start=False, stop=True)
                    pA = psum.tile([P, D], f32, tag="pA")
                    nt = T_lo[b] + T_hi[b]
                    lo_off = sum(T_lo[ch['b0']:b])
                    hi_off = sum(T_hi[ch['b0']:b])
                    tb = tile_base[b]
                    # all sel matrices of the block in one DVE op
                    selb = work.tile([P, selmax, P], bf, tag="sel")
                    nc.vector.tensor_tensor(
                        out=selb[:, :nt, :],
                        in0=slots[:, tb:tb + nt]
                            .rearrange("p (k o) -> p k o", o=1)
                            .to_broadcast([P, nt, P]),
                        in1=iota[:].rearrange("p (o d) -> p o d", o=1)
                            .to_broadcast([P, nt, P]),
                        op=mybir.AluOpType.is_equal)
                    # self-loop contribution: psum += I @ g_sb[block]
                    nc.tensor.matmul(out=pA[:], lhsT=ident_bf[:],
                                     rhs=g_own_l[:, b * D:(b + 1) * D],
                                     start=True, stop=False)
                    for t in range(nt):
                        if t < T_lo[b]:
                            tch = lo_off + t
                            if tch < ta:
                                src = gloa[:, tch, :]
                            else:
                                src = glob[:, tch - ta, :]
                        else:
                            src = ghi[:, hi_off + (t - T_lo[b]), :]
                        nc.tensor.matmul(out=pA[:], lhsT=selb[:, t, :], rhs=src,
                                         start=False, stop=(t == nt - 1))
                    t1 = outp.tile([P, D], f32, tag="t1")
                    nc.scalar.activation(out=t1[:], in_=pA[:],
                                         func=mybir.ActivationFunctionType.Copy,
                                         scale=dinv[:, b:b + 1])
                    t2 = outp.tile([P, D], f32, tag="t2")
                    nc.vector.tensor_tensor(out=t2[:], in0=t1[:], in1=pB[:],
                                            op=mybir.AluOpType.add)
                    xo = outp.tile([P, D], f32, tag="xo")
                    nc.scalar.activation(out=xo[:], in_=t2[:],
                                         func=mybir.ActivationFunctionType.Relu)
                    if l == 0:
                        pT = psum.tile([P, D], f32, tag="pT")
                        nc.tensor.transpose(out=pT[:], in_=xo[:], identity=ident[:])
                        nc.vector.tensor_copy(out=x1T[:, cs], in_=pT[:])
                        ph2 = psum.tile([P, D], f32, tag="ph")
                        nc.tensor.matmul(out=ph2[:], lhsT=x1T[:, cs], rhs=w2b[:],
                                         start=True, stop=True)
                        nc.scalar.activation(out=g2sb[:, b * D:(b + 1) * D],
                                             in_=ph2[:],
                                             func=mybir.ActivationFunctionType.Copy,
                                             scale=dinv[:, b:b + 1])
                        p2, lb2 = piece_of(b)
                        nc.sync.dma_start(out=g2p[p2][lb2 * P:(lb2 + 1) * P, :],
                                          in_=g2sb[:, b * D:(b + 1) * D])
                    else:
                        nc.sync.dma_start(out=out_p[cs, :], in_=xo[:])
            return ag2_emitted

        with nc.named_scope("layer1"):
            em = layer(0, g1_full, g1sb)
            for p in range(NPIECE):
                if p not in em:
                    ag_piece("ag2", g2p, g2_full, p)
        with nc.named_scope("layer2"):
            layer(1, g2_full, g2sb)
    return nc


# ---------------------------------------------------------------------------
# Entry point
# ---------------------------------------------------------------------------

def make_inputs(x, conv_w, conv_b, res_w, res_b, meta):
    npc, npad = meta['npc'], meta['npad']
    iota = np.tile(np.arange(P, dtype=np.float32), (P, 1)).astype(bf16)
    in_maps = []
    for c in range(C):
        xT = np.zeros((P, npad), dtype=np.float32)
        xT[:, :npc] = np.asarray(x[c * npc:(c + 1) * npc], dtype=np.float32).T
        in_maps.append({
            "xT": xT,
            "w1": np.asarray(conv_w[0], dtype=np.float32),
            "w2": np.asarray(conv_w[1], dtype=np.float32),
            "resw": np.asarray(res_w, dtype=np.float32),
            "convb": np.asarray(conv_b, dtype=np.float32),
            "resb": np.asarray(res_b, dtype=np.float32).reshape(1, D),
            "deg": meta['deg'][c],
            "idx_lo": meta['idx_lo'][c],
            "idx_hi": meta['idx_hi'][c],
            "slot": meta['slots'][c].astype(bf16),
            "iota": iota,
            "ident": np.eye(P, dtype=np.float32),
            "ones": np.ones((1, D), dtype=np.float32).astype(bf16),
        })
    return in_maps


def run(x, edge_index, conv_w, conv_b, res_w, res_b, trace=False, trace_kwargs=None):
    N = x.shape[0]
    meta = prep(edge_index, N)
    nc = build_program(meta)
    nc.compile()
    split_sync_waits(nc)
    in_maps = make_inputs(x, conv_w, conv_b, res_w, res_b, meta)
    res = run_bass_kernel_spmd(nc, in_maps, list(range(C)), trace=trace,
                               **(trace_kwargs or {}))
    npc = meta['npc']
    out = np.concatenate([np.asarray(res.results[c]["out"])[:npc]
                          for c in range(C)], axis=0)
    return out.astype(np.float32), res


def kernel(x, edge_index, conv_w, conv_b, res_w, res_b):
    out, _ = run(x, edge_index, conv_w, conv_b, res_w, res_b, trace=False)
    return out



# revision 24
# speedup vs baseline: 1.0657x; 1.0657x over previous
"""GCN 2-layer message-passing kernel for Trainium2 (8 NeuronCores, Bass/Tile).

Strategy (graph/data parallel per the sharding hint):
  - Nodes partitioned into 8 contiguous ranges (6250 per core).
  - Host does INTEGER/index prep only: add self-loops, bucket edges by
    destination core/block-of-128, sort by source, build gather-index +
    dst-slot metadata, integer in-degree counts. All floating-point math
    runs on device.
  - Per layer: each core computes g = dinv * (x @ W) rows for its own
    nodes (PE matmul + ACT per-partition scale, cast to bf16), AllGather
    of the bf16 g-table across the 8 cores (halo exchange). Then, per
    chunk of destination blocks, one dma_gather instruction fetches all
    message rows (g[src]) for the chunk's edge tiles; per 128-edge tile a
    0/1 selection matrix (DVE is_equal against an iota matrix) scatter-
    reduces the messages into the block's PSUM accumulator via one PE
    matmul; the block output is
      relu(dinv * segsum + x @ res_w + conv_b + res_b)
    with bias via a K=1 outer-product matmul and residual accumulated in
    a second PSUM bank; dst-degree scaling via ACT per-partition scale.
  - dma_gather uses int16 indices, so the gather table is addressed in a
    low half (rows < 32768) and a high half; each block's edges are
    split into low/high tile groups (host-side integer split).

kernel(**inputs) takes FULL inputs and returns the FULL [50000, 128]
float32 output.
"""
import sys
from contextlib import ExitStack

import numpy as np

if '/opt/trn_rl_repo' not in sys.path:
    sys.path.insert(0, '/opt/trn_rl_repo')

import ml_dtypes

from concourse import bacc, mybir, tile
from concourse.bass_utils import run_bass_kernel_spmd
from concourse.vector_clock import ScopedClock


def _patched_drain_and_barrier(self, tick_clock, wait_clock):
    """Split the kernel-tail drain's sem waits across single-wait drains:
    walrus's NO_STRUCT codegen rejects >1 sync wait on InstDrain."""
    drain_inst = self.nc.sync.drain()
    wait_clock.add_sem_waits(drain_inst.ins,
                             ScopedClock({None: tick_clock.global_clock}))
    si = drain_inst.ins.sync_info
    if si is not None and si.on_wait is not None and len(si.on_wait) > 1:
        waits = list(si.on_wait)
        del si.on_wait[1:]
        for w in waits[1:]:
            d2 = self.nc.sync.drain()
            si2 = d2.ins.sync_info
            if si2 is None:
                d2.ins.sync_info = mybir.SyncInfo(on_wait=[w], on_update=[])
            else:
                si2.on_wait.append(w)
    self.nc.all_engine_barrier()
    assert self.sems is not None
    popped = self.nc._tile_sem_poison_stack.pop()
    assert popped is self._sem_poison
    self.nc.clear_and_free_semaphores(list(self.sems.allocated().values()))
    self.nc.all_engine_barrier()


tile.TileContext._drain_and_barrier = _patched_drain_and_barrier


def split_sync_waits(nc, max_waits=1):
    """Walrus codegen rejects >1 sync wait on several instruction encodings.
    Hoist excess waits onto same-engine no-ops placed just before."""
    import bass_rust
    try:
        funcs = list(nc.m.functions)
    except Exception:
        funcs = [nc.main_func]
    seen = 0
    for fn in funcs:
        for bb in fn.blocks:
            insts = bb.instructions
            new = []
            for ins in insts:
                si = ins.sync_info
                if si is not None and si.on_wait and len(si.on_wait) > max_waits:
                    waits = list(si.on_wait)
                    extra, keep = waits[:-max_waits], waits[-max_waits:]
                    for w in extra:
                        nop = bass_rust.InstNoOp(
                            name=f"I-waitsplit-{seen}", ins=[], outs=[])
                        seen += 1
                        nop.engine = ins.engine
                        nop.sync_info = mybir.SyncInfo(on_wait=[w], on_update=[])
                        new.append(nop)
                    del si.on_wait[:]
                    si.on_wait.extend(keep)
                new.append(ins)
            insts[:] = new
    return seen


bf16 = ml_dtypes.bfloat16
P = 128          # partitions / tile edge
C = 8            # cores
D = 128          # hidden dim
HI = 32768       # int16 index reach of dma_gather
CB = 4           # dst blocks per gather chunk


# ---------------------------------------------------------------------------
# Host-side integer/index prep (sharding + metadata; no FP math on values)
# ---------------------------------------------------------------------------

def prep(edge_index, n_nodes):
    N = n_nodes
    npc = N // C
    assert npc * C == N
    B = (npc + P - 1) // P
    npad = B * P

    ei = np.asarray(edge_index)
    # self-loops are handled on-device via an identity matmul per block;
    # they still count toward the degree.
    src_all = ei[0].astype(np.int64)
    dst_all = ei[1].astype(np.int64)
    deg_all = np.bincount(dst_all, minlength=N) + 1

    own_s = src_all // npc
    local = src_all - own_s * npc
    # piece-major table layout: piece p = blocks [PBH[p], PBH[p+1]) of every
    # core, so each piece's AllGather lands contiguously in the table.
    PBH = [0, 13, 25, 37, B]
    pb_rows = [(PBH[p + 1] - PBH[p]) * P for p in range(4)]
    base = np.concatenate([[0], np.cumsum([C * r for r in pb_rows])])
    blk_of = local >> 7
    piece = np.searchsorted(np.array(PBH[1:]), blk_of, side='right')
    row_all = (base[piece] + own_s * np.array(pb_rows)[piece]
               + (local - np.array(PBH)[piece] * P))

    owner_all = dst_all // npc
    per_core = []
    nlo = np.zeros((C, B), dtype=np.int64)
    nhi = np.zeros((C, B), dtype=np.int64)
    for c in range(C):
        m = owner_all == c
        r = row_all[m]
        dloc = dst_all[m] - c * npc
        blk = dloc >> 7
        slot = dloc & 127
        hi = (r >= HI).astype(np.int64)
        # sort by (block, hi, row) so each block = [lo edges..., hi edges...]
        order = np.lexsort((r, hi, blk))
        r, blk, slot, hi = r[order], blk[order], slot[order], hi[order]
        per_core.append((r, blk, slot))
        for b in range(B):
            mb = blk == b
            nhi[c, b] = (hi[mb]).sum()
            nlo[c, b] = mb.sum() - nhi[c, b]

    T_lo = np.maximum((nlo.max(axis=0) + P - 1) // P, 1)
    T_hi = (nhi.max(axis=0) + P - 1) // P            # may be 0 for a block
    T_b = (T_lo + T_hi).astype(np.int64)
    T_total = int(T_b.sum())

    # per-block tile layout: T_lo[b] low tiles then T_hi[b] high tiles
    tile_base = np.concatenate([[0], np.cumsum(T_b)])
    # low/high gather index sequences (tile-major, per core)
    n_lo_total = int(T_lo.sum()) * P
    n_hi_total = int(T_hi.sum()) * P
    lo_base = np.concatenate([[0], np.cumsum(T_lo)])   # in tiles
    hi_base = np.concatenate([[0], np.cumsum(T_hi)])

    slots = np.full((C, T_total * P), -1.0, dtype=np.float32)
    idx_lo = np.zeros((C, n_lo_total), dtype=np.int64)
    idx_hi = np.zeros((C, max(n_hi_total, 16)), dtype=np.int64)
    for c in range(C):
        r, blk, slot = per_core[c]
        bstart = np.concatenate([[0], np.cumsum(nlo[c] + nhi[c])])
        for b in range(B):
            e0, e1 = bstart[b], bstart[b + 1]
            k_lo = int(nlo[c, b])
            rl, sl = r[e0:e0 + k_lo], slot[e0:e0 + k_lo]
            rh, sh = r[e0 + k_lo:e1], slot[e0 + k_lo:e1]
            o = lo_base[b] * P
            idx_lo[c, o:o + k_lo] = rl
            o = hi_base[b] * P
            idx_hi[c, o:o + len(rh)] = rh - HI
            o = tile_base[b] * P
            slots[c, o:o + k_lo] = sl
            o2 = (tile_base[b] + T_lo[b]) * P
            slots[c, o2:o2 + len(sh)] = sh

    deg = np.ones((C, P, B), dtype=np.float32)
    sdeg_row = np.ones((C, 1, npad), dtype=np.float32)
    for c in range(C):
        dpad = np.ones(npad, dtype=np.float32)
        dpad[:npc] = deg_all[c * npc:(c + 1) * npc].astype(np.float32)
        deg[c] = dpad.reshape(B, P).T
        sdeg_row[c, 0] = np.sqrt(dpad)

    def pack16(a):
        # wrapped layout: element j -> [j % 16, j // 16], replicated to the
        # 8 Q7 cores' partition groups (128 partitions total)
        n = a.shape[1]
        w = a.reshape(a.shape[0], n // 16, 16).transpose(0, 2, 1).astype(np.int16)
        return np.tile(w, (1, 8, 1)).copy()

    # chunking of blocks for gather calls
    chunks = []
    for b0 in range(0, B, CB):
        b1 = min(b0 + CB, B)
        chunks.append(dict(
            b0=b0, b1=b1,
            lo_t0=int(lo_base[b0]), lo_t1=int(lo_base[b1]),
            hi_t0=int(hi_base[b0]), hi_t1=int(hi_base[b1]),
        ))

    return dict(
        npc=npc, npad=npad, B=B,
        T_lo=T_lo.tolist(), T_hi=T_hi.tolist(), T_b=T_b.tolist(),
        tile_base=tile_base.tolist(), T_total=T_total,
        n_lo16=n_lo_total // 16, n_hi16=max(n_hi_total, 16) // 16,
        chunks=chunks,
        idx_lo=pack16(idx_lo), idx_hi=pack16(idx_hi),
        slots=slots.reshape(C, T_total, P).transpose(0, 2, 1).copy(),
        deg=deg, sdeg_row=sdeg_row,
    )


# ---------------------------------------------------------------------------
# Device program (uniform across the 8 cores)
# ---------------------------------------------------------------------------

def build_program(meta):
    npad, B, T_total = meta['npad'], meta['B'], meta['T_total']
    T_lo, T_hi, tile_base = meta['T_lo'], meta['T_hi'], meta['tile_base']
    chunks = meta['chunks']
    TBL = C * npad
    f32 = mybir.dt.float32
    bf = mybir.dt.bfloat16
    max_lo_tiles = max(ch['lo_t1'] - ch['lo_t0'] for ch in chunks)
    max_hi_tiles = max(ch['hi_t1'] - ch['hi_t0'] for ch in chunks)

    nc = bacc.Bacc(None, target_bir_lowering=False, num_swdge_queues=4)
    xT_p = nc.declare_dram_parameter("xT", [P, npad], f32, isOutput=False)
    w1_p = nc.declare_dram_parameter("w1", [P, D], f32, isOutput=False)
    w2_p = nc.declare_dram_parameter("w2", [P, D], f32, isOutput=False)
    rw_p = nc.declare_dram_parameter("resw", [P, D], f32, isOutput=False)
    cb_p = nc.declare_dram_parameter("convb", [2, D], f32, isOutput=False)
    rb_p = nc.declare_dram_parameter("resb", [1, D], f32, isOutput=False)
    deg_p = nc.declare_dram_parameter("deg", [P, B], f32, isOutput=False)
    ilo_p = nc.declare_dram_parameter("idx_lo", [128, meta['n_lo16']], mybir.dt.int16, isOutput=False)
    ihi_p = nc.declare_dram_parameter("idx_hi", [128, meta['n_hi16']], mybir.dt.int16, isOutput=False)
    slot_p = nc.declare_dram_parameter("slot", [P, T_total], bf, isOutput=False)
    iota_p = nc.declare_dram_parameter("iota", [P, P], bf, isOutput=False)
    ident_p = nc.declare_dram_parameter("ident", [P, P], f32, isOutput=False)
    ones_p = nc.declare_dram_parameter("ones", [1, D], bf, isOutput=False)
    out_p = nc.declare_dram_parameter("out", [npad, D], f32, isOutput=True)

    # AllGather in 4 pieces so transfers overlap compute; piece tensors give
    # tile exact deps, outs land strided in the single gather table.
    PB = [0, 13, 25, 37, B]
    NPIECE = 4
    g1p = [nc.dram_tensor(f"g1o{p}", [(PB[p + 1] - PB[p]) * P, D], bf)
           for p in range(NPIECE)]
    g2p = [nc.dram_tensor(f"g2o{p}", [(PB[p + 1] - PB[p]) * P, D], bf)
           for p in range(NPIECE)]
    g1_full = nc.dram_tensor("g1_full", [TBL, D], bf, addr_space="Shared")
    g2_full = nc.dram_tensor("g2_full", [TBL, D], bf, addr_space="Shared")

    def piece_of(b):
        for p in range(NPIECE):
            if PB[p] <= b < PB[p + 1]:
                return p, b - PB[p]
        raise AssertionError

    pb_rows = [(PB[p + 1] - PB[p]) * P for p in range(NPIECE)]
    pbase = [0]
    for p in range(NPIECE):
        pbase.append(pbase[-1] + C * pb_rows[p])

    def ag_piece(tag, gp_list, g_full, p):
        with nc.named_scope(f"{tag}_{p}"):
            nc.gpsimd.collective_compute(
                "AllGather", mybir.AluOpType.bypass,
                replica_groups=[list(range(C))],
                ins=[gp_list[p][:, :]],
                outs=[g_full[pbase[p]:pbase[p + 1], :]])

    with tile.TileContext(nc) as tc, ExitStack() as ctx:
        const = ctx.enter_context(tc.tile_pool(name="const", bufs=1))
        gbufL = ctx.enter_context(tc.tile_pool(name="gbufL", bufs=6))
        gbufH = ctx.enter_context(tc.tile_pool(name="gbufH", bufs=3))
        work = ctx.enter_context(tc.tile_pool(name="work", bufs=4))
        outp = ctx.enter_context(tc.tile_pool(name="outp", bufs=3))
        psum = ctx.enter_context(tc.tile_pool(name="psum", bufs=2, space="PSUM"))

        # ---- constants / persistent state ----
        xT = const.tile([P, npad], bf)
        for k0 in range(0, npad, 784):
            xstg = outp.tile([P, 784], f32, tag="xstg")
            nc.sync.dma_start(out=xstg[:], in_=xT_p[:, k0:k0 + 784])
            nc.vector.tensor_copy(out=xT[:, k0:k0 + 784], in_=xstg[:])
        x1T = const.tile([P, npad], bf)          # layer-1 output, transposed
        ones1 = const.tile([1, D], bf)
        nc.sync.dma_start(out=ones1[:], in_=ones_p[:, :])
        w1f = const.tile([P, D], f32)
        nc.sync.dma_start(out=w1f[:], in_=w1_p[:, :])
        w1 = const.tile([P, D], bf)
        nc.vector.tensor_copy(out=w1[:], in_=w1f[:])
        w2f = const.tile([P, D], f32)
        nc.sync.dma_start(out=w2f[:], in_=w2_p[:, :])
        rwf = const.tile([P, D], f32)
        nc.sync.dma_start(out=rwf[:], in_=rw_p[:, :])
        w2b = const.tile([P, D], bf)
        nc.vector.tensor_copy(out=w2b[:], in_=w2f[:])
        rwb = const.tile([P, D], bf)
        nc.vector.tensor_copy(out=rwb[:], in_=rwf[:])
        # SBUF-resident scaled g tables for the self-loop contribution
        g1sb = const.tile([P, B * D], bf)
        g2sb = const.tile([P, B * D], bf)

        rb = const.tile([1, D], f32)
        nc.sync.dma_start(out=rb[:], in_=rb_p[:, :])
        bcomb = []
        for l in range(2):
            cbl = const.tile([1, D], f32, tag=f"cb{l}")
            nc.sync.dma_start(out=cbl[:], in_=cb_p[l:l + 1, :])
            bc = const.tile([1, D], bf, tag=f"bcomb{l}")
            nc.vector.tensor_tensor(out=bc[:], in0=cbl[:], in1=rb[:],
                                    op=mybir.AluOpType.add)
            bcomb.append(bc)

        iota = const.tile([P, P], bf)
        nc.sync.dma_start(out=iota[:], in_=iota_p[:, :])
        ident = const.tile([P, P], f32)
        nc.sync.dma_start(out=ident[:], in_=ident_p[:, :])
        ident_bf = const.tile([P, P], bf)
        nc.vector.tensor_copy(out=ident_bf[:], in_=ident[:])

        ilo = const.tile([128, meta['n_lo16']], mybir.dt.int16)
        nc.sync.dma_start(out=ilo[:], in_=ilo_p[:, :])
        ihi = const.tile([128, meta['n_hi16']], mybir.dt.int16)
        nc.sync.dma_start(out=ihi[:], in_=ihi_p[:, :])
        slots = const.tile([P, T_total], bf)
        nc.sync.dma_start(out=slots[:], in_=slot_p[:, :])

        degt = const.tile([P, B], f32)
        nc.sync.dma_start(out=degt[:], in_=deg_p[:, :])
        sdeg = const.tile([P, B], f32)
        nc.scalar.activation(out=sdeg[:], in_=degt[:],
                             func=mybir.ActivationFunctionType.Sqrt)
        dinv = const.tile([P, B], f32)
        nc.vector.reciprocal(out=dinv[:], in_=sdeg[:])


        # ---- phase 1: g1 = dinv * (x @ W1) for own rows, then AllGather ----
        with nc.named_scope("phase1"):
            for b in range(B):
                cs = slice(b * P, (b + 1) * P)
                ds = slice(b * D, (b + 1) * D)
                ph = psum.tile([P, D], f32, tag="ph")
                nc.tensor.matmul(out=ph[:], lhsT=xT[:, cs], rhs=w1[:],
                                 start=True, stop=True)
                nc.scalar.activation(out=g1sb[:, ds], in_=ph[:],
                                     func=mybir.ActivationFunctionType.Copy,
                                     scale=dinv[:, b:b + 1])
                p, lb = piece_of(b)
                nc.sync.dma_start(out=g1p[p][lb * P:(lb + 1) * P, :],
                                  in_=g1sb[:, ds])
                if b == PB[p + 1] - 1:
                    ag_piece("ag1", g1p, g1_full, p)

        # greedy queue balancing: 4 SWDGE queues = 4 independent Q7 core
        # pairs generating gather descriptors in parallel
        qload = [0, 0, 0, 0]

        def pick_queue(nidx):
            q = qload.index(min(qload))
            qload[q] += nidx
            return q

        def layer(l, g_full, g_own_l):
            selmax = max(T_lo[b] + T_hi[b] for b in range(B))
            half_lo = (max_lo_tiles + 1) // 2
            # queue position of each ag2 piece: ~3 chunks after its last block
            # is produced, so the Pool queue never stalls waiting for it
            ag2_emitted = set()
            ag2_at = {}
            if l == 0:
                for p in range(NPIECE):
                    ag2_at.setdefault((PB[p + 1] - 1) // CB + 2, []).append(p)
            for ci, ch in enumerate(chunks):
                for p in ag2_at.get(ci, []):
                    ag_piece("ag2", g2p, g2_full, p)
                    ag2_emitted.add(p)
                nlo_t = ch['lo_t1'] - ch['lo_t0']
                nhi_t = ch['hi_t1'] - ch['hi_t0']
                # split the lo gather across two queues for deeper overlap
                ta = (nlo_t + 1) // 2
                tb = nlo_t - ta
                gloa = gbufL.tile([P, half_lo, D], bf, tag="gloa")
                nc.gpsimd.dma_gather(
                    out_ap=gloa[:, :ta, :], in_ap=g_full[:, :],
                    idxs_ap=ilo[:, ch['lo_t0'] * 8:(ch['lo_t0'] + ta) * 8],
                    num_idxs=ta * P, num_idxs_reg=ta * P, elem_size=D,
                    single_packet=False, queue_num=pick_queue(ta * P))
                glob = gbufL.tile([P, half_lo, D], bf, tag="glob")
                if tb > 0:
                    nc.gpsimd.dma_gather(
                        out_ap=glob[:, :tb, :], in_ap=g_full[:, :],
                        idxs_ap=ilo[:, (ch['lo_t0'] + ta) * 8:ch['lo_t1'] * 8],
                        num_idxs=tb * P, num_idxs_reg=tb * P, elem_size=D,
                        single_packet=False, queue_num=pick_queue(tb * P))
                if nhi_t > 0:
                    ghi = gbufH.tile([P, max(max_hi_tiles, 1), D], bf, tag="ghi")
                    nc.gpsimd.dma_gather(
                        out_ap=ghi[:, :nhi_t, :], in_ap=g_full[HI:, :],
                        idxs_ap=ihi[:, ch['hi_t0'] * 8:ch['hi_t1'] * 8],
                        num_idxs=nhi_t * P, num_idxs_reg=nhi_t * P, elem_size=D,
                        single_packet=False, queue_num=pick_queue(nhi_t * P))
                for b in range(ch['b0'], ch['b1']):
                    cs = slice(b * P, (b + 1) * P)
                    # sqrt(deg)-scaled bias + residual in their own bank so
                    # the PE can run them independently of the scatter chain
                    pB = psum.tile([P, D], f32, tag="pB")
                    nc.tensor.matmul(out=pB[:], lhsT=ones1[:],
                                     rhs=bcomb[l][:], start=True, stop=False)
                    nc.tensor.matmul(out=pB[:],
                                     lhsT=(xT if l == 0 else x1T)[:, cs],
                                     rhs=rwb[:], start=False, stop=True)
                    pA = psum.tile([P, D], f32, tag="pA")
                    nt = T_lo[b] + T_hi[b]
                    lo_off = sum(T_lo[ch['b0']:b])
                    hi_off = sum(T_hi[ch['b0']:b])
                    tb = tile_base[b]
                    # all sel matrices of the block in one DVE op
                    selb = work.tile([P, selmax, P], bf, tag="sel")
                    nc.vector.tensor_tensor(
                        out=selb[:, :nt, :],
                        in0=slots[:, tb:tb + nt]
                            .rearrange("p (k o) -> p k o", o=1)
                            .to_broadcast([P, nt, P]),
                        in1=iota[:].rearrange("p (o d) -> p o d", o=1)
                            .to_broadcast([P, nt, P]),
                        op=mybir.AluOpType.is_equal)
                    # self-loop contribution: psum += I @ g_sb[block]
                    nc.tensor.matmul(out=pA[:], lhsT=ident_bf[:],
                                     rhs=g_own_l[:, b * D:(b + 1) * D],
                                     start=True, stop=False)
                    for t in range(nt):
                        if t < T_lo[b]:
                            tch = lo_off + t
                            if tch < ta:
                                src = gloa[:, tch, :]
                            else:
                                src = glob[:, tch - ta, :]
                        else:
                            src = ghi[:, hi_off + (t - T_lo[b]), :]
                        nc.tensor.matmul(out=pA[:], lhsT=selb[:, t, :], rhs=src,
                                         start=False, stop=(t == nt - 1))
                    t1 = outp.tile([P, D], f32, tag="t1")
                    nc.scalar.activation(out=t1[:], in_=pA[:],
                                         func=mybir.ActivationFunctionType.Copy,
                                         scale=dinv[:, b:b + 1])
                    t2 = outp.tile([P, D], f32, tag="t2")
                    nc.vector.tensor_tensor(out=t2[:], in0=t1[:], in1=pB[:],
                                            op=mybir.AluOpType.add)
                    xo = outp.tile([P, D], f32, tag="xo")
                    nc.scalar.activation(out=xo[:], in_=t2[:],
                                         func=mybir.ActivationFunctionType.Relu)
                    if l == 0:
                        pT = psum.tile([P, D], f32, tag="pT")
                        nc.tensor.transpose(out=pT[:], in_=xo[:], identity=ident[:])
                        nc.vector.tensor_copy(out=x1T[:, cs], in_=pT[:])
                        ph2 = psum.tile([P, D], f32, tag="ph")
                        nc.tensor.matmul(out=ph2[:], lhsT=x1T[:, cs], rhs=w2b[:],
                                         start=True, stop=True)
                        nc.scalar.activation(out=g2sb[:, b * D:(b + 1) * D],
                                             in_=ph2[:],
                                             func=mybir.ActivationFunctionType.Copy,
                                             scale=dinv[:, b:b + 1])
                        p2, lb2 = piece_of(b)
                        nc.sync.dma_start(out=g2p[p2][lb2 * P:(lb2 + 1) * P, :],
                                          in_=g2sb[:, b * D:(b + 1) * D])
                    else:
                        nc.sync.dma_start(out=out_p[cs, :], in_=xo[:])
            return ag2_emitted

        with nc.named_scope("layer1"):
            em = layer(0, g1_full, g1sb)
            for p in range(NPIECE):
                if p not in em:
                    ag_piece("ag2", g2p, g2_full, p)
        with nc.named_scope("layer2"):
            layer(1, g2_full, g2sb)
    return nc


# ---------------------------------------------------------------------------
# Entry point
# ---------------------------------------------------------------------------

def make_inputs(x, conv_w, conv_b, res_w, res_b, meta):
    npc, npad = meta['npc'], meta['npad']
    iota = np.tile(np.arange(P, dtype=np.float32), (P, 1)).astype(bf16)
    in_maps = []
    for c in range(C):
        xT = np.zeros((P, npad), dtype=np.float32)
        xT[:, :npc] = np.asarray(x[c * npc:(c + 1) * npc], dtype=np.float32).T
        in_maps.append({
            "xT": xT,
            "w1": np.asarray(conv_w[0], dtype=np.float32),
            "w2": np.asarray(conv_w[1], dtype=np.float32),
            "resw": np.asarray(res_w, dtype=np.float32),
            "convb": np.asarray(conv_b, dtype=np.float32),
            "resb": np.asarray(res_b, dtype=np.float32).reshape(1, D),
            "deg": meta['deg'][c],
            "idx_lo": meta['idx_lo'][c],
            "idx_hi": meta['idx_hi'][c],
            "slot": meta['slots'][c].astype(bf16),
            "iota": iota,
            "ident": np.eye(P, dtype=np.float32),
            "ones": np.ones((1, D), dtype=np.float32).astype(bf16),
        })
    return in_maps


def run(x, edge_index, conv_w, conv_b, res_w, res_b, trace=False, trace_kwargs=None):
    N = x.shape[0]
    meta = prep(edge_index, N)
    nc = build_program(meta)
    nc.compile()
    split_sync_waits(nc)
    in_maps = make_inputs(x, conv_w, conv_b, res_w, res_b, meta)
    res = run_bass_kernel_spmd(nc, in_maps, list(range(C)), trace=trace,
                               **(trace_kwargs or {}))
    npc = meta['npc']
    out = np.concatenate([np.asarray(res.results[c]["out"])[:npc]
                          for c in range(C)], axis=0)
    return out.astype(np.float32), res


def kernel(x, edge_index, conv_w, conv_b, res_w, res_b):
    out, _ = run(x, edge_index, conv_w, conv_b, res_w, res_b, trace=False)
    return out

